# revision 3
# baseline (speedup 1.0000x reference)
"""Cosine-similarity loss kernel for Trainium2 (8 NeuronCores, SPMD).

loss = -sum_n dot(s_n, im_n) / (||s_n|| * ||im_n||)   for s, im in R^{65536 x 512}

Strategy (memory-bound problem, ~360 GB/s HBM per core):
  - Shard the 65536 rows across 8 cores (8192 rows each, 32 MB/core to stream).
  - Per core: stream [128, 8, 512] f32 tiles (2 MB per tensor) via HWDGE DMA.
  - Per 128-row slice, three fused one-pass reductions:
      dot  = sum_d s*im   -> VectorE tensor_tensor_reduce
      ii   = sum_d im*im  -> VectorE tensor_tensor_reduce
      ss   = sum_d s*s    -> ScalarE activation(Square, accum_out=...)
    DVE ~1.19us/slice-pair + ACT ~0.72us stay under the 1.42us DMA floor.
  - Tiny tail: loss_p[128,1] = -sum_slices dot/sqrt(ss*ii); DMA out.
  - Host sums the 8x128 partials and returns -total as f32 scalar.
"""

import numpy as np
from contextlib import ExitStack

import concourse.bacc as bacc
import concourse.bass as bass
import concourse.mybir as mybir
import concourse.tile as tile
from concourse.bass_utils import run_bass_kernel_spmd

N, D = 65536, 512
N_CORES = 8
ROWS = N // N_CORES          # 8192 rows per core
P = 128                      # SBUF partitions
F32 = mybir.dt.float32


def _build(rows=ROWS, a=8, bufs=3):
    """Build + compile the per-core Bass module. `a` = 128-row slices per DMA tile."""
    slices = rows // P       # reductions per core
    tiles = slices // a      # DMA tiles per core
    assert tiles * a == slices and slices * P == rows

    nc = bacc.Bacc(
        "TRN2", target_bir_lowering=False, debug=False, num_devices=N_CORES
    )
    s_d = nc.dram_tensor("s", [rows, D], F32, kind="ExternalInput").ap()
    im_d = nc.dram_tensor("im", [rows, D], F32, kind="ExternalInput").ap()
    out_d = nc.dram_tensor("out", [P, 1], F32, kind="ExternalOutput").ap()

    # row = (t*a + j)*128 + p  ->  tile t, free dims (j, d), partition p
    s_r = s_d.rearrange("(t j p) d -> t p j d", p=P, j=a)
    im_r = im_d.rearrange("(t j p) d -> t p j d", p=P, j=a)

    mult = mybir.AluOpType.mult
    add = mybir.AluOpType.add

    with tile.TileContext(nc) as tc, ExitStack() as ctx:
        spool = ctx.enter_context(tc.tile_pool(name="spool", bufs=bufs))
        ipool = ctx.enter_context(tc.tile_pool(name="ipool", bufs=bufs))
        stats = ctx.enter_context(tc.tile_pool(name="stats", bufs=1))

        dot_all = stats.tile([P, slices], F32)
        ss_all = stats.tile([P, slices], F32)
        ii_all = stats.tile([P, slices], F32)
        dve_scr = stats.tile([P, D], F32)
        act_scr = stats.tile([P, D], F32)

        for t in range(tiles):
            st = spool.tile([P, a, D], F32, name="st")
            nc.sync.dma_start(st[:], s_r[t])
            it = ipool.tile([P, a, D], F32, name="it")
            nc.sync.dma_start(it[:], im_r[t])
            for j in range(a):
                c = t * a + j
                nc.vector.scalar_tensor_tensor(
                    out=dve_scr[:], in0=st[:, j, :], scalar=1.0, in1=it[:, j, :],
                    op0=mult, op1=mult,
                    accum_out=dot_all[:, c : c + 1],
                )
                nc.vector.scalar_tensor_tensor(
                    out=dve_scr[:], in0=it[:, j, :], scalar=1.0, in1=it[:, j, :],
                    op0=mult, op1=mult,
                    accum_out=ii_all[:, c : c + 1],
                )
                nc.scalar.activation(
                    out=act_scr[:], in_=st[:, j, :],
                    func=mybir.ActivationFunctionType.Square,
                    accum_out=ss_all[:, c : c + 1],
                )

        # tail: loss_p = -sum_c dot_c / sqrt(ss_c * ii_c)
        prod = stats.tile([P, slices], F32)
        nc.vector.tensor_tensor(out=prod[:], in0=ss_all[:], in1=ii_all[:], op=mult)
        nrm = stats.tile([P, slices], F32)
        nc.scalar.activation(nrm[:], prod[:], mybir.ActivationFunctionType.Sqrt)
        rcp = stats.tile([P, slices], F32)
        nc.vector.reciprocal(rcp[:], nrm[:])
        fin_scr = stats.tile([P, slices], F32)
        loss_p = stats.tile([P, 1], F32)
        nc.vector.scalar_tensor_tensor(
            out=fin_scr[:], in0=dot_all[:], scalar=-1.0, in1=rcp[:],
            op0=mult, op1=mult,
            accum_out=loss_p[:],
        )
        nc.sync.dma_start(out_d, loss_p[:])

    nc.compile()
    return nc


_compiled = None


def _get_nc():
    global _compiled
    if _compiled is None:
        _compiled = _build()
    return _compiled


def _run(s, im, **kw):
    """Shard, run on 8 cores, return (per-core results, BassKernelResults)."""
    s = np.ascontiguousarray(np.asarray(s, dtype=np.float32))
    im = np.ascontiguousarray(np.asarray(im, dtype=np.float32))
    assert s.shape == (N, D) and im.shape == (N, D)
    nc = _get_nc()
    in_maps = [
        {"s": s[c * ROWS : (c + 1) * ROWS], "im": im[c * ROWS : (c + 1) * ROWS]}
        for c in range(N_CORES)
    ]
    bkr = run_bass_kernel_spmd(nc, in_maps, core_ids=list(range(N_CORES)), **kw)
    return bkr


def kernel(s, im, temp=None, **_):
    bkr = _run(s, im)
    total = np.float64(0.0)
    for r in bkr.results:
        total += r["out"].astype(np.float64).sum()
    return np.float32(total)
